# revision 34
# baseline (speedup 1.0000x reference)
"""Multi-head causal self-attention on 8 Trainium2 NeuronCores.

Problem: x[2, 2048, 2048], 16 heads x 128 dim, causal softmax, four
2048x2048 projections (nn.Linear convention y = x @ W.T).

Sharding: head tensor-parallel. Core c owns heads {2c, 2c+1}: it computes
those heads' Q/K/V projections, per-head causal attention, and the slice of
the output projection that consumes those heads (Wo columns 256c..256c+256).
Each core emits a full-shape partial output; the host sums the 8 partials.

Per-core kernel (all matmuls via the PE with fp32r operands):
  phase 0: transpose W slices once (PE identity transpose) -> W^T resident.
  phase 1: per 512-token block: transpose x tiles -> x^T; accumulate
           Q^T/K^T ([d, tok], per head) and V ([tok, d]) in PSUM.
  phase 2: per (head, q-block of 512): scores^T chunks [k=128, q=512]
           = K^T_chunk.T-contraction vs Q^T (causal chunks only), additive
           causal mask on diagonal chunks, exp on ACT (scale = 1/sqrt(d),
           no max-subtraction: scores are O(1)), Z row via ones-matmul,
           O^T = V.T @ E^T accumulated over k chunks, normalized by 1/Z.
  phase 3: output projection from O^T per head into one PSUM accumulator,
           DMA'd straight from PSUM to DRAM.
"""

from contextlib import ExitStack

import numpy as np

import concourse.bacc as bacc
import concourse.mybir as mybir
import concourse.tile as tile
from concourse.bass_utils import run_bass_kernel_spmd
from concourse.masks import make_identity

N_CORES = 8
B = 2
SEQ = 2048
H = 2048
NHEADS = 16
D = 128
HPC = NHEADS // N_CORES  # heads per core
DC = HPC * D             # per-core head dims (256)
QB = 512                 # q/token block (moving free dim)
KTH = H // 128           # 16 contraction tiles over hidden
MASK_NEG = -1.0e5
SCALE = 1.0 / float(np.sqrt(D))

F32 = mybir.dt.float32
F32R = mybir.dt.float32r

EXP = mybir.ActivationFunctionType.Exp


def build(seq=SEQ, reps=1, use_f32r=True, only=None, bufs=None):
    """Emit the per-core program. seq is parameterized for small dev runs."""
    mdt = F32R if use_f32r else F32
    t = B * seq
    nblocks = seq // QB          # token blocks per batch
    nchunks = seq // 128         # 128-token chunks per batch

    nc = bacc.Bacc("TRN2", target_bir_lowering=False, debug=False,
                   num_devices=N_CORES)
    x_ap = nc.dram_tensor("x", [t, H], F32, kind="ExternalInput").ap()
    wq_ap = nc.dram_tensor("wq", [DC, H], F32, kind="ExternalInput").ap()
    wk_ap = nc.dram_tensor("wk", [DC, H], F32, kind="ExternalInput").ap()
    wv_ap = nc.dram_tensor("wv", [DC, H], F32, kind="ExternalInput").ap()
    wo_ap = nc.dram_tensor("wo", [H, DC], F32, kind="ExternalInput").ap()
    out_ap = nc.dram_tensor("out", [t, H], F32, kind="ExternalOutput").ap()

    with tile.TileContext(nc) as tc, ExitStack() as ctx:
        const = ctx.enter_context(tc.tile_pool(name="const", bufs=1))
        ident = const.tile([128, 128], F32, name="ident")
        make_identity(nc, ident[:])
        ones_f32 = const.tile([128, 1], F32, name="ones_f32")
        nc.gpsimd.memset(ones_f32[:], 1.0)
        ones = const.tile([128, 1], mdt, name="ones")
        nc.vector.tensor_copy(ones[:], ones_f32[:])
        # [1, 128] row of ones: stationary for the z-broadcast outer product
        ones_row_f32 = const.tile([1, 128], F32, name="ones_row_f32")
        nc.gpsimd.memset(ones_row_f32[:], 1.0)
        ones_row = const.tile([1, 128], mdt, name="ones_row")
        nc.vector.tensor_copy(ones_row[:], ones_row_f32[:])
        # Multiplicative causal mask: maskw[p, w] = 1 if (w - p - 384) >= 0
        # else 0. Diagonal chunk with k0 - q0 = 128*(3-m) uses cols
        # [384-128m, +512); applied to E^T after exp.
        maskw = const.tile([128, 896], F32, name="maskw")
        nc.gpsimd.memset(maskw[:], 1.0)
        nc.gpsimd.affine_select(
            out=maskw[:], in_=maskw[:],
            compare_op=mybir.AluOpType.is_ge,
            fill=0.0, base=-384,
            pattern=[[1, 896]], channel_multiplier=-1,
        )

        # --- phase 0: W transposes (resident) ---
        wt_pool = ctx.enter_context(tc.tile_pool(name="wt", bufs=1))
        # w{q,k,v}T[h_loc, kt*DC + r*128 + d] = W[r*128 + d, kt*128 + h_loc]
        wqkvT = {
            nm: wt_pool.tile([128, KTH * DC], mdt, name=f"w{nm}T", tag=f"w{nm}T")
            for nm in ("q", "k", "v")
        }
        # woT[d_loc, hh*H + o] = Wo[o, hh*128 + d_loc]
        woT = wt_pool.tile([128, HPC * H], mdt, name="woT", tag="woT")

        # PSUM: 4 one-bank slots + 2 two-bank slots = 8 banks
        pspool = ctx.enter_context(tc.tile_pool(name="ps", bufs=4, space="PSUM"))
        ps2pool = ctx.enter_context(tc.tile_pool(name="ps2", bufs=2,
                                                 space="PSUM"))

        with tc.tile_pool(name="wload", bufs=2) as wload:
            for nm, w_ap in (("q", wq_ap), ("k", wk_ap), ("v", wv_ap)):
                wt_v = wqkvT[nm][:].rearrange("p (k dc) -> p k dc", dc=DC)
                for r in range(HPC):
                    wnat = wload.tile([128, H], F32, tag="wload", name="wnat")
                    nc.sync.dma_start(wnat[:], w_ap[r * 128:(r + 1) * 128, :])
                    for q4 in range(KTH // 4):
                        ps = pspool.tile([128, 512], F32, tag="ps", name="pst")
                        for j in range(4):
                            kt = q4 * 4 + j
                            nc.tensor.transpose(
                                ps[:, j * 128:(j + 1) * 128],
                                wnat[:, kt * 128:(kt + 1) * 128], ident[:])
                        nc.vector.tensor_copy(
                            wt_v[:, q4 * 4:(q4 + 1) * 4, r * 128:r * 128 + 128],
                            ps[:].rearrange("p (k t) -> p k t", t=128))
            for ot in range(H // 128):
                wonat = wload.tile([128, H], F32, tag="wload", name="wonat")
                nc.sync.dma_start(
                    wonat[:, :DC], wo_ap[ot * 128:(ot + 1) * 128, :])
                for hh in range(HPC):
                    ps = pspool.tile([128, 128], F32, tag="ps", name="pst")
                    nc.tensor.transpose(
                        ps[:], wonat[:, hh * 128:(hh + 1) * 128], ident[:])
                    col = hh * H + ot * 128
                    nc.vector.tensor_copy(woT[:, col:col + 128], ps[:])

        # --- main: per batch, projections then attention ---
        bd = {"et": 3, "xnat": 8, "z": 2, "otb": 1, "stage": 4}
        bd.update(bufs or {})
        qkv_pool = ctx.enter_context(tc.tile_pool(name="qkv", bufs=1))
        xt_pool = ctx.enter_context(tc.tile_pool(name="xt", bufs=1))
        et_pool = ctx.enter_context(tc.tile_pool(name="et", bufs=bd["et"]))
        xnat_pool = ctx.enter_context(tc.tile_pool(name="xnat", bufs=bd["xnat"]))
        z_pool = ctx.enter_context(tc.tile_pool(name="z", bufs=bd["z"]))
        ot_pool = ctx.enter_context(tc.tile_pool(name="otb", bufs=bd["otb"]))
        stage_pool = ctx.enter_context(tc.tile_pool(name="stage", bufs=bd["stage"]))

        def body():
            for b in range(B):
                qt_sb = [qkv_pool.tile([128, seq], mdt, tag=f"qt{h}", name=f"qt{h}")
                         for h in range(HPC)]
                kt_sb = [qkv_pool.tile([128, seq], mdt, tag=f"kt{h}", name=f"kt{h}")
                        for h in range(HPC)]
                vn_sb = qkv_pool.tile([128, nchunks * DC], mdt, tag="vn", name="vn")
                if only == 'attn':
                    # profiling-only mode: give the attention phase defined
                    # inputs without emitting the projections
                    for tb in qt_sb + kt_sb + [vn_sb]:
                        nc.vector.memset(tb[:].bitcast(F32), 0.0)

                # phase 1: Q^T/K^T [d, tok], V [tok, d] for this batch
                for nb in range(nblocks) if only != 'attn' else []:
                    tok0 = b * seq + nb * QB
                    xt_sb = xt_pool.tile([128, KTH * QB], mdt, tag="xt",
                                         name="xt")
                    xt_v = xt_sb[:].rearrange("p (k t) -> p k t", t=QB)
                    xts = [xt_sb[:, kt * QB:(kt + 1) * QB]
                           for kt in range(KTH)]
                    for c4 in range(QB // 128):
                        for hq in range(2):
                            kt0 = hq * 8
                            ps = ps2pool.tile([128, 1024], F32, tag="ps2",
                                              name="pst")
                            for quar in range(2):
                                xn = xnat_pool.tile([128, 512], F32,
                                                    tag="xnat", name="xn")
                                col0 = (kt0 + quar * 4) * 128
                                nc.sync.dma_start(
                                    xn[:],
                                    x_ap[tok0 + c4 * 128:
                                         tok0 + (c4 + 1) * 128,
                                         col0:col0 + 512])
                                for j in range(4):
                                    nc.tensor.transpose(
                                        ps[:, (quar * 4 + j) * 128:
                                           (quar * 4 + j + 1) * 128],
                                        xn[:, j * 128:(j + 1) * 128],
                                        ident[:])
                            nc.vector.tensor_copy(
                                xt_v[:, kt0:kt0 + 8,
                                     c4 * 128:(c4 + 1) * 128],
                                ps[:].rearrange("p (k t) -> p k t", t=128))
                    for nm_p, dst in (("q", qt_sb), ("k", kt_sb)):
                        acc = [pspool.tile([128, QB], F32, tag="ps",
                                           name="pacc") for _ in range(HPC)]
                        for kt in range(KTH):
                            first, last = kt == 0, kt == KTH - 1
                            for hh in range(HPC):
                                col = kt * DC + hh * 128
                                nc.tensor.matmul(
                                    acc[hh][:],
                                    (wqkvT[nm_p][:, col:col + 128]),
                                    (xts[kt][:]), start=first, stop=last)
                        for hh in range(HPC):
                            nc.vector.tensor_copy(
                                dst[hh][:, nb * QB:(nb + 1) * QB], acc[hh][:])
                    for c4 in range(QB // 128):
                        vn_ps = pspool.tile([128, DC], F32, tag="ps", name="vnps")
                        for kt in range(KTH):
                            nc.tensor.matmul(
                                vn_ps[:],
                                (xts[kt][:, c4 * 128:(c4 + 1) * 128]),
                                (wqkvT["v"][:, kt * DC:(kt + 1) * DC]),
                                start=(kt == 0), stop=(kt == KTH - 1))
                        chunk = nb * (QB // 128) + c4
                        nc.scalar.copy(
                            vn_sb[:, chunk * DC:(chunk + 1) * DC], vn_ps[:])

                # phase 2+3: attention + output projection per q block
                for qb in range(nblocks) if only != 'proj' else []:
                    q0 = qb * QB
                    n_kc = (qb + 1) * (QB // 128)
                    ot_sbs = []
                    for hh in range(HPC):
                        # Interleave scores/exp with the Z and AV consumers so
                        # each E^T tile's lifetime spans only a couple of k
                        # chunks (bounded et pool). Chunks are processed in
                        # PAIRS: two score matmuls land in the two banks of
                        # one [128, 1024] PSUM tile and share one exp call
                        # (the ACT fixed overhead is ~40% of a 512-col call).
                        def score_pair(pc):
                            st_ps = ps2pool.tile([128, 2 * QB], F32,
                                                 tag="ps2", name="stps")
                            et2 = et_pool.tile([128, 2 * QB], mdt, tag="et",
                                               name="et")
                            for half in range(2):
                                kc = 2 * pc + half
                                nc.tensor.matmul(
                                    st_ps[:, half * QB:(half + 1) * QB],
                                    (kt_sb[hh][:, kc * 128:(kc + 1) * 128]),
                                    (qt_sb[hh][:, q0:q0 + QB]),
                                    start=True, stop=True)
                            nc.scalar.activation(et2[:], st_ps[:], EXP,
                                                 scale=SCALE)
                            for half in range(2):
                                kc = 2 * pc + half
                                m = kc - (n_kc - 4)
                                if m >= 0:
                                    off = 384 - 128 * m
                                    sl = et2[:, half * QB:(half + 1) * QB]
                                    nc.vector.tensor_mul(
                                        sl, sl, maskw[:, off:off + QB])
                            return et2

                        n_pc = n_kc // 2
                        z_ps = pspool.tile([1, QB], F32, tag="ps", name="zps")
                        ot_ps = pspool.tile([128, QB], F32, tag="ps",
                                            name="otps")
                        ets = {pc: score_pair(pc)
                               for pc in range(min(2, n_pc))}
                        for kc in range(n_kc):
                            pc = kc // 2
                            if kc % 2 == 0 and pc + 2 < n_pc:
                                ets[pc + 2] = score_pair(pc + 2)
                            et2 = ets[pc]
                            et = et2[:, (kc % 2) * QB:(kc % 2 + 1) * QB]
                            if kc % 2 == 1:
                                ets.pop(pc)
                            first, last = kc == 0, kc == n_kc - 1
                            nc.tensor.matmul(
                                z_ps[:], (ones[:]), (et[:]),
                                start=first, stop=last)
                            col = kc * DC + hh * 128
                            nc.tensor.matmul(
                                ot_ps[:], (vn_sb[:, col:col + 128]),
                                (et[:]),
                                start=first, stop=last)
                        z_sb = z_pool.tile([1, QB], mdt, tag="zrow", name="zrow")
                        nc.vector.tensor_copy(z_sb[:], z_ps[:])
                        zb_ps = pspool.tile([128, QB], F32, tag="ps",
                                            name="zbps")
                        nc.tensor.matmul(zb_ps[:], (ones_row[:]), (z_sb[:]),
                                         start=True, stop=True)
                        zb = z_pool.tile([128, QB], F32, tag="zb", name="zb")
                        nc.vector.reciprocal(zb[:], zb_ps[:])
                        ot_sb = ot_pool.tile([128, QB], mdt, tag=f"ot{hh}", name=f"ot{hh}")
                        nc.vector.tensor_mul(ot_sb[:], ot_ps[:], zb[:])
                        ot_sbs.append(ot_sb)
                    for c4 in range(QB // 128):
                        for oc2 in range(H // (2 * QB)):
                            stg = stage_pool.tile([128, 2 * QB], F32,
                                                  tag="stage", name="stg")
                            for half in range(2):
                                oc = oc2 * 2 + half
                                op_ps = pspool.tile([128, QB], F32, tag="ps",
                                                    name="opps")
                                for hh in range(HPC):
                                    nc.tensor.matmul(
                                        op_ps[:],
                                        (ot_sbs[hh][:,
                                                    c4 * 128:(c4 + 1) * 128]),
                                        (woT[:, hh * H + oc * QB:
                                               hh * H + (oc + 1) * QB]),
                                        start=(hh == 0), stop=(hh == HPC - 1))
                                nc.vector.tensor_copy(
                                    stg[:, half * QB:(half + 1) * QB],
                                    op_ps[:])
                            row0 = b * seq + q0 + c4 * 128
                            nc.sync.dma_start(
                                out_ap[row0:row0 + 128,
                                       oc2 * 2 * QB:(oc2 + 1) * 2 * QB],
                                stg[:])

        if reps == 1:
            body()
        else:
            with tc.For_i(0, reps, 1):
                body()

    nc.compile()
    return nc


def shard_inputs(x, Wq, Wk, Wv, Wo, seq=SEQ):
    t = B * seq
    x2 = np.ascontiguousarray(np.asarray(x, dtype=np.float32).reshape(t, H))
    Wq = np.asarray(Wq, dtype=np.float32)
    Wk = np.asarray(Wk, dtype=np.float32)
    Wv = np.asarray(Wv, dtype=np.float32)
    Wo = np.asarray(Wo, dtype=np.float32)
    in_maps = []
    for c in range(N_CORES):
        sl = slice(c * DC, (c + 1) * DC)
        in_maps.append({
            "x": x2,
            "wq": np.ascontiguousarray(Wq[sl, :]),
            "wk": np.ascontiguousarray(Wk[sl, :]),
            "wv": np.ascontiguousarray(Wv[sl, :]),
            "wo": np.ascontiguousarray(Wo[:, sl]),
        })
    return in_maps


_cache = {}


def kernel(x, Wq, Wk, Wv, Wo):
    if "nc" not in _cache:
        _cache["nc"] = build()
    nc = _cache["nc"]
    in_maps = shard_inputs(x, Wq, Wk, Wv, Wo)
    res = run_bass_kernel_spmd(nc, in_maps, list(range(N_CORES)))
    acc = res.results[0]["out"].astype(np.float32)
    for c in range(1, N_CORES):
        acc = acc + res.results[c]["out"]
    return acc.reshape(B, SEQ, H)
